# revision 18
# baseline (speedup 1.0000x reference)
"""Multi-head attention (B=2, S=2048, E=1024, H=16) on 8 TRN2 NeuronCores.

Sharding: batch x head-group. Core c handles batch b = c // 4 and the
4 heads (256 features) of group g = c % 4. Each core computes its
q/k/v projections (column-sharded weights), transposed-layout attention
(scores kept as [j, i] so exp(scores) feeds the P@V matmul directly as
the moving operand), and a partial output projection against its row
slice of Wo^T. Host sums the 4 partials per batch and folds in the
bv/bo biases (exact: softmax rows sum to 1, so bv contributes
bv @ Wo.T; bk is softmax-invariant and dropped; bq and the 1/sqrt(Dh)
scale are folded into Wq/bq host-side).

Matmuls run as float32r (full-rate fp32 on the PE with TF32-like
mantissa rounding); softmax runs unshifted (scores are O(5), safe in
fp32) with the row sum obtained for free by augmenting V with a ones
column. exp() runs on the scalar engine out of PSUM; normalization is a
DVE multiply against a gpsimd partition-broadcast of 1/Z.
"""

import math
import os

import ml_dtypes
import numpy as np

import concourse.bass as bass
from concourse import bacc
import concourse.mybir as mybir
import concourse.tile as tile
from concourse.bass_utils import run_bass_kernel_spmd

B, S, E, H = 2, 2048, 1024, 16
Dh = E // H  # 64
NCORES = 8
GPB = NCORES // B  # head-groups (cores) per batch
HPC = H // GPB  # heads per core
F = HPC * Dh  # 256 features per core
FC = F // 128  # 2 f-chunks
EC = E // 128  # 8 e-chunks
SB = 512  # s-block (projection/out-proj N)
NSB = S // SB
NST = S // 128  # 16 s-tiles / j-chunks
IB = 512  # attention i-block
NIB = S // IB
F32 = mybir.dt.float32
F32R = mybir.dt.float32r
BF16 = mybir.dt.bfloat16

# exec time (ns) of the last traced run; test.py reads this.
last_exec_time_ns = None
last_results = None

_built = None


def _build():
    nc = bacc.Bacc()
    xq = nc.dram_tensor("xq", [E, S], F32R, kind="ExternalInput")  # query[b].T
    xk = nc.dram_tensor("xk", [E, S], F32R, kind="ExternalInput")  # key[b].T
    xv = nc.dram_tensor("xv", [E, S], F32R, kind="ExternalInput")  # value[b].T
    wqt = nc.dram_tensor("wqt", [E, F], F32R, kind="ExternalInput")
    wkt = nc.dram_tensor("wkt", [E, F], F32R, kind="ExternalInput")
    wvt = nc.dram_tensor("wvt", [E, F], F32R, kind="ExternalInput")
    wot = nc.dram_tensor("wot", [F, E], BF16, kind="ExternalInput")
    bqv = nc.dram_tensor("bqv", [128, FC], F32, kind="ExternalInput")
    out = nc.dram_tensor("out", [S, E], F32, kind="ExternalOutput")

    with tile.TileContext(nc) as tc:
        with (
            tc.tile_pool(name="wpool", bufs=1) as wpool,
            tc.tile_pool(name="persist", bufs=1) as persist,
            tc.tile_pool(name="xpool", bufs=3) as xpool,
            tc.tile_pool(name="epool", bufs=8) as epool,
            tc.tile_pool(name="spool", bufs=4) as spool,
            tc.tile_pool(name="opool", bufs=2) as opool,
            tc.tile_pool(name="pp", bufs=2, space="PSUM") as pp,
            tc.tile_pool(name="stp", bufs=2, space="PSUM") as stp,
            tc.tile_pool(name="atp", bufs=2, space="PSUM") as atp,
        ):
            wq_sb = wpool.tile([128, EC, F], F32R, name="wq_sb")
            nc.sync.dma_start(wq_sb[:], wqt.rearrange("(ec p) f -> p ec f", p=128))
            wk_sb = wpool.tile([128, EC, F], F32R, name="wk_sb")
            nc.sync.dma_start(wk_sb[:], wkt.rearrange("(ec p) f -> p ec f", p=128))
            wv_sb = wpool.tile([128, EC, F], F32R, name="wv_sb")
            nc.sync.dma_start(wv_sb[:], wvt.rearrange("(ec p) f -> p ec f", p=128))
            wo_sb = wpool.tile([128, FC, E], BF16, name="wo_sb")
            nc.sync.dma_start(wo_sb[:], wot.rearrange("(fc p) n -> p fc n", p=128))
            bq_sb = wpool.tile([128, FC], F32, name="bq_sb")
            nc.sync.dma_start(bq_sb[:], bqv[:, :])

            qT = persist.tile([128, FC, S], BF16, name="qT")
            kT = persist.tile([128, FC, S], BF16, name="kT")
            vA = persist.tile([128, NST, HPC, Dh + 1], BF16, name="vA")
            aT = persist.tile([128, FC, S], BF16, name="aT")
            nc.vector.memset(vA[:, :, :, Dh : Dh + 1].bitcast(mybir.dt.uint16), 0x3F80)

            # --- q projection helper (emitted per i-block, earliest) ---
            xq_r = xq.rearrange("(ec p) s -> p ec s", p=128)

            def q_proj(ib):
                isl = slice(ib * IB, (ib + 1) * IB)
                xt = xpool.tile([128, EC, SB], F32R, tag="xT", name="xt")
                nc.sync.dma_start(xt[:], xq_r[:, :, isl])
                for fc in range(FC):
                    ps = pp.tile([128, SB], F32, tag="ps", name="ps")
                    for ec in range(EC):
                        nc.tensor.matmul(
                            ps[:],
                            wq_sb[:, ec, fc * 128 : (fc + 1) * 128],
                            xt[:, ec, :],
                            start=(ec == 0),
                            stop=(ec == EC - 1),
                        )
                    nc.vector.tensor_scalar_add(
                        qT[:, fc, isl], ps[:], bq_sb[:, fc : fc + 1]
                    )

            q_proj(0)

            # --- k projection: kT[f, s] = Wk.T slice @ x.T ---
            xk_r = xk.rearrange("(ec p) s -> p ec s", p=128)
            for sb in range(NSB):
                xt = xpool.tile([128, EC, SB], F32R, tag="xT", name="xt")
                nc.sync.dma_start(xt[:], xk_r[:, :, sb * SB : (sb + 1) * SB])
                for fc in range(FC):
                    ps = pp.tile([128, SB], F32, tag="ps", name="ps")
                    for ec in range(EC):
                        nc.tensor.matmul(
                            ps[:],
                            wk_sb[:, ec, fc * 128 : (fc + 1) * 128],
                            xt[:, ec, :],
                            start=(ec == 0),
                            stop=(ec == EC - 1),
                        )
                    nc.vector.tensor_copy(kT[:, fc, sb * SB : (sb + 1) * SB], ps[:])

            # --- v projection: v[s, f] (s on partitions), ones column at f=Dh ---
            xv_r = xv.rearrange("(ec p) s -> p ec s", p=128)
            for sb in range(NSB):
                xt = xpool.tile([128, EC, SB], F32R, tag="xT", name="xt")
                nc.sync.dma_start(xt[:], xv_r[:, :, sb * SB : (sb + 1) * SB])
                for st in range(SB // 128):
                    jt = sb * (SB // 128) + st
                    psv = pp.tile([128, F], F32, tag="ps", name="psv")
                    for ec in range(EC):
                        nc.tensor.matmul(
                            psv[:],
                            xt[:, ec, st * 128 : (st + 1) * 128],
                            wv_sb[:, ec, :],
                            start=(ec == 0),
                            stop=(ec == EC - 1),
                        )
                    nc.vector.tensor_copy(
                        vA[:, jt, :, 0:Dh],
                        psv[:].rearrange("p (h d) -> p h d", h=HPC),
                    )

            # --- per i-block: attention (head-pairs packed into PE row
            #     groups); q-proj of the next block leads each segment and
            #     out-proj trails by one block to keep the PE fed while the
            #     DVE/gpsimd normalize chain drains ---

            def out_proj(ib):
                for st in range(ib * (IB // 128), (ib + 1) * (IB // 128)):
                    ob = opool.tile([128, E], F32, tag="ob", name="ob")
                    for n2 in range(E // SB):
                        pso = pp.tile([128, SB], F32, tag="ps", name="pso")
                        for fc in range(FC):
                            nc.tensor.matmul(
                                pso[:],
                                aT[:, fc, st * 128 : (st + 1) * 128],
                                wo_sb[:, fc, n2 * SB : (n2 + 1) * SB],
                                start=(fc == 0),
                                stop=(fc == FC - 1),
                            )
                        nc.vector.tensor_copy(ob[:, n2 * SB : (n2 + 1) * SB], pso[:])
                    nc.sync.dma_start(out[st * 128 : (st + 1) * 128, :], ob[:])

            for ib in range(NIB):
                isl = slice(ib * IB, (ib + 1) * IB)
                if ib + 1 < NIB:
                    q_proj(ib + 1)
                for fc in range(FC):
                    if fc == 1 and ib > 0:
                        out_proj(ib - 1)
                    ats = [
                        atp.tile([Dh + 1, IB], F32, tag="at", name=f"at{t}")
                        for t in range(2)
                    ]
                    for jc in range(NST):
                        jsl = slice(jc * 128, (jc + 1) * 128)
                        st_ps = stp.tile([128, 2, IB], F32, tag="st", name="st")
                        nc.tensor.matmul(
                            st_ps[:, 0, :],
                            kT[0:Dh, fc, jsl],
                            qT[0:Dh, fc, isl],
                            start=True,
                            stop=True,
                            tile_position=(0, 0),
                        )
                        nc.tensor.matmul(
                            st_ps[:, 1, :],
                            kT[Dh:128, fc, jsl],
                            qT[Dh:128, fc, isl],
                            start=True,
                            stop=True,
                            tile_position=(64, 0),
                        )
                        e = epool.tile([128, 2, IB], BF16, tag="E", name="e")
                        nc.scalar.activation(
                            e[:], st_ps[:], mybir.ActivationFunctionType.Exp
                        )
                        for t in range(2):
                            nc.tensor.matmul(
                                ats[t][:],
                                vA[:, jc, 2 * fc + t, :],
                                e[:, t, :],
                                start=(jc == 0),
                                stop=(jc == NST - 1),
                            )
                    atcs = []
                    for t in range(2):
                        atc = spool.tile([Dh + 1, IB], F32, tag="atc", name="atc")
                        nc.vector.tensor_copy(atc[:], ats[t][:])
                        atcs.append(atc)
                    for t in range(2):
                        po = t * Dh
                        rz = spool.tile([1, IB], F32, tag="rz", name="rz")
                        nc.vector.reciprocal(rz[:], atcs[t][Dh : Dh + 1, :])
                        bc = spool.tile([Dh, IB], F32, tag="bc", name="bc")
                        nc.gpsimd.partition_broadcast(bc[:], rz[:])
                        nc.gpsimd.tensor_tensor(
                            aT[po : po + Dh, fc, isl],
                            atcs[t][0:Dh, :],
                            bc[:],
                            mybir.AluOpType.mult,
                        )

            out_proj(NIB - 1)

    nc.compile()
    return nc


def kernel(key, query, value, Wq, bq, Wk, bk, Wv, bv, Wo, bo):
    global last_exec_time_ns, last_results, _built
    key = np.asarray(key, np.float32)
    query = np.asarray(query, np.float32)
    value = np.asarray(value, np.float32)
    Wq = np.asarray(Wq, np.float32)
    Wk = np.asarray(Wk, np.float32)
    Wv = np.asarray(Wv, np.float32)
    Wo = np.asarray(Wo, np.float32)
    bq = np.asarray(bq, np.float32)
    bv = np.asarray(bv, np.float32)
    bo = np.asarray(bo, np.float32)

    s = 1.0 / math.sqrt(Dh)
    xqT = [np.ascontiguousarray(query[b].T) for b in range(B)]
    xkT = [np.ascontiguousarray(key[b].T) for b in range(B)]
    xvT = [np.ascontiguousarray(value[b].T) for b in range(B)]
    WqTs = np.ascontiguousarray(Wq.T) * s
    WkT = np.ascontiguousarray(Wk.T)
    WvT = np.ascontiguousarray(Wv.T)
    WoT = np.ascontiguousarray(Wo.T)

    in_maps = []
    for c in range(NCORES):
        b, g = c // GPB, c % GPB
        fsl = slice(g * F, (g + 1) * F)
        in_maps.append(
            {
                "xq": xqT[b],
                "xk": xkT[b],
                "xv": xvT[b],
                "wqt": np.ascontiguousarray(WqTs[:, fsl]),
                "wkt": np.ascontiguousarray(WkT[:, fsl]),
                "wvt": np.ascontiguousarray(WvT[:, fsl]),
                "wot": np.ascontiguousarray(WoT[fsl, :]).astype(ml_dtypes.bfloat16),
                "bqv": np.ascontiguousarray((bq[fsl] * s).reshape(FC, 128).T),
            }
        )

    if _built is None:
        _built = _build()
    trace = bool(int(os.environ.get("KERNEL_TRACE", "0")))
    res = run_bass_kernel_spmd(
        _built, in_maps, core_ids=list(range(NCORES)), trace=trace
    )
    last_exec_time_ns = res.exec_time_ns
    last_results = res
    parts = [r["out"] for r in res.results]

    outs = np.stack(
        [sum(parts[b * GPB + g] for g in range(GPB)) for b in range(B)]
    ).astype(np.float32)
    outs += (bv @ Wo.T + bo)[None, None, :]
    return outs


# revision 19
# speedup vs baseline: 1.0658x; 1.0658x over previous
"""Multi-head attention (B=2, S=2048, E=1024, H=16) on 8 TRN2 NeuronCores.

Sharding: batch x head-group. Core c handles batch b = c // 4 and the
4 heads (256 features) of group g = c % 4. Each core computes its
q/k/v projections (column-sharded weights), transposed-layout attention
(scores kept as [j, i] so exp(scores) feeds the P@V matmul directly as
the moving operand), and a partial output projection against its row
slice of Wo^T. Host sums the 4 partials per batch and folds in the
bv/bo biases (exact: softmax rows sum to 1, so bv contributes
bv @ Wo.T; bk is softmax-invariant and dropped; bq and the 1/sqrt(Dh)
scale are folded into Wq/bq host-side).

Matmuls run as float32r (full-rate fp32 on the PE with TF32-like
mantissa rounding); softmax runs unshifted (scores are O(5), safe in
fp32) with the row sum obtained for free by augmenting V with a ones
column. exp() runs on the scalar engine out of PSUM; normalization is a
DVE multiply against a gpsimd partition-broadcast of 1/Z.
"""

import math
import os

import ml_dtypes
import numpy as np

import concourse.bass as bass
from concourse import bacc
import concourse.mybir as mybir
import concourse.tile as tile
from concourse.bass_utils import run_bass_kernel_spmd

B, S, E, H = 2, 2048, 1024, 16
Dh = E // H  # 64
NCORES = 8
GPB = NCORES // B  # head-groups (cores) per batch
HPC = H // GPB  # heads per core
F = HPC * Dh  # 256 features per core
FC = F // 128  # 2 f-chunks
EC = E // 128  # 8 e-chunks
SB = 512  # s-block (projection/out-proj N)
NSB = S // SB
NST = S // 128  # 16 s-tiles / j-chunks
IB = 512  # attention i-block
NIB = S // IB
F32 = mybir.dt.float32
F32R = mybir.dt.float32r
BF16 = mybir.dt.bfloat16

# exec time (ns) of the last traced run; test.py reads this.
last_exec_time_ns = None
last_results = None

_built = None


def _build():
    nc = bacc.Bacc()
    xq = nc.dram_tensor("xq", [E, S], F32R, kind="ExternalInput")  # query[b].T
    xk = nc.dram_tensor("xk", [E, S], F32R, kind="ExternalInput")  # key[b].T
    xv = nc.dram_tensor("xv", [E, S], F32R, kind="ExternalInput")  # value[b].T
    wqt = nc.dram_tensor("wqt", [E, F], F32R, kind="ExternalInput")
    wkt = nc.dram_tensor("wkt", [E, F], F32R, kind="ExternalInput")
    wvt = nc.dram_tensor("wvt", [E, F], F32R, kind="ExternalInput")
    wot = nc.dram_tensor("wot", [F, E], BF16, kind="ExternalInput")
    bqv = nc.dram_tensor("bqv", [1, F], F32R, kind="ExternalInput")
    out = nc.dram_tensor("out", [S, E], F32, kind="ExternalOutput")

    with tile.TileContext(nc) as tc:
        with (
            tc.tile_pool(name="wpool", bufs=1) as wpool,
            tc.tile_pool(name="persist", bufs=1) as persist,
            tc.tile_pool(name="xpool", bufs=3) as xpool,
            tc.tile_pool(name="epool", bufs=8) as epool,
            tc.tile_pool(name="spool", bufs=4) as spool,
            tc.tile_pool(name="opool", bufs=2) as opool,
            tc.tile_pool(name="pp", bufs=2, space="PSUM") as pp,
            tc.tile_pool(name="stp", bufs=2, space="PSUM") as stp,
            tc.tile_pool(name="atp", bufs=2, space="PSUM") as atp,
        ):
            wq_sb = wpool.tile([128, EC, F], F32R, name="wq_sb")
            nc.sync.dma_start(wq_sb[:], wqt.rearrange("(ec p) f -> p ec f", p=128))
            wk_sb = wpool.tile([128, EC, F], F32R, name="wk_sb")
            nc.sync.dma_start(wk_sb[:], wkt.rearrange("(ec p) f -> p ec f", p=128))
            wv_sb = wpool.tile([128, EC, F], F32R, name="wv_sb")
            nc.sync.dma_start(wv_sb[:], wvt.rearrange("(ec p) f -> p ec f", p=128))
            wo_sb = wpool.tile([128, FC, E], BF16, name="wo_sb")
            nc.sync.dma_start(wo_sb[:], wot.rearrange("(fc p) n -> p fc n", p=128))
            bq_sb = wpool.tile([1, F], F32R, name="bq_sb")
            nc.sync.dma_start(bq_sb[:], bqv[:, :])
            ones1 = wpool.tile([1, SB], F32R, name="ones1")
            nc.vector.memset(ones1[:].bitcast(F32), 1.0)

            qT = persist.tile([128, FC, S], BF16, name="qT")
            kT = persist.tile([128, FC, S], BF16, name="kT")
            vA = persist.tile([128, NST, HPC, Dh + 1], BF16, name="vA")
            aT = persist.tile([128, FC, S], BF16, name="aT")
            nc.vector.memset(vA[:, :, :, Dh : Dh + 1].bitcast(mybir.dt.uint16), 0x3F80)

            # --- q projection helper (emitted per i-block, earliest) ---
            xq_r = xq.rearrange("(ec p) s -> p ec s", p=128)

            def q_proj(ib):
                isl = slice(ib * IB, (ib + 1) * IB)
                xt = xpool.tile([128, EC, SB], F32R, tag="xT", name="xt")
                nc.sync.dma_start(xt[:], xq_r[:, :, isl])
                for fc in range(FC):
                    ps = pp.tile([128, SB], F32, tag="ps", name="ps")
                    for ec in range(EC):
                        nc.tensor.matmul(
                            ps[:],
                            wq_sb[:, ec, fc * 128 : (fc + 1) * 128],
                            xt[:, ec, :],
                            start=(ec == 0),
                            stop=False,
                        )
                    nc.tensor.matmul(
                        ps[:],
                        bq_sb[0:1, fc * 128 : (fc + 1) * 128],
                        ones1[0:1, :],
                        start=False,
                        stop=True,
                    )
                    nc.scalar.activation(
                        qT[:, fc, isl], ps[:], mybir.ActivationFunctionType.Copy
                    )

            q_proj(0)

            # --- k projection: kT[f, s] = Wk.T slice @ x.T ---
            xk_r = xk.rearrange("(ec p) s -> p ec s", p=128)
            for sb in range(NSB):
                xt = xpool.tile([128, EC, SB], F32R, tag="xT", name="xt")
                nc.sync.dma_start(xt[:], xk_r[:, :, sb * SB : (sb + 1) * SB])
                for fc in range(FC):
                    ps = pp.tile([128, SB], F32, tag="ps", name="ps")
                    for ec in range(EC):
                        nc.tensor.matmul(
                            ps[:],
                            wk_sb[:, ec, fc * 128 : (fc + 1) * 128],
                            xt[:, ec, :],
                            start=(ec == 0),
                            stop=(ec == EC - 1),
                        )
                    nc.vector.tensor_copy(kT[:, fc, sb * SB : (sb + 1) * SB], ps[:])

            # --- v projection: v[s, f] (s on partitions), ones column at f=Dh ---
            xv_r = xv.rearrange("(ec p) s -> p ec s", p=128)
            for sb in range(NSB):
                xt = xpool.tile([128, EC, SB], F32R, tag="xT", name="xt")
                nc.sync.dma_start(xt[:], xv_r[:, :, sb * SB : (sb + 1) * SB])
                for st in range(SB // 128):
                    jt = sb * (SB // 128) + st
                    psv = pp.tile([128, F], F32, tag="ps", name="psv")
                    for ec in range(EC):
                        nc.tensor.matmul(
                            psv[:],
                            xt[:, ec, st * 128 : (st + 1) * 128],
                            wv_sb[:, ec, :],
                            start=(ec == 0),
                            stop=(ec == EC - 1),
                        )
                    nc.vector.tensor_copy(
                        vA[:, jt, :, 0:Dh],
                        psv[:].rearrange("p (h d) -> p h d", h=HPC),
                    )

            # --- per i-block: attention (head-pairs packed into PE row
            #     groups); q-proj of the next block leads each segment and
            #     out-proj trails by one block to keep the PE fed while the
            #     DVE/gpsimd normalize chain drains ---

            def out_proj(ib):
                for st in range(ib * (IB // 128), (ib + 1) * (IB // 128)):
                    ob = opool.tile([128, E], F32, tag="ob", name="ob")
                    for n2 in range(E // SB):
                        pso = pp.tile([128, SB], F32, tag="ps", name="pso")
                        for fc in range(FC):
                            nc.tensor.matmul(
                                pso[:],
                                aT[:, fc, st * 128 : (st + 1) * 128],
                                wo_sb[:, fc, n2 * SB : (n2 + 1) * SB],
                                start=(fc == 0),
                                stop=(fc == FC - 1),
                            )
                        nc.vector.tensor_copy(ob[:, n2 * SB : (n2 + 1) * SB], pso[:])
                    nc.sync.dma_start(out[st * 128 : (st + 1) * 128, :], ob[:])

            for ib in range(NIB):
                isl = slice(ib * IB, (ib + 1) * IB)
                for fc in range(FC):
                    ats = [
                        atp.tile([Dh + 1, IB], F32, tag="at", name=f"at{t}")
                        for t in range(2)
                    ]
                    for jc in range(NST):
                        jsl = slice(jc * 128, (jc + 1) * 128)
                        st_ps = stp.tile([128, 2, IB], F32, tag="st", name="st")
                        nc.tensor.matmul(
                            st_ps[:, 0, :],
                            kT[0:Dh, fc, jsl],
                            qT[0:Dh, fc, isl],
                            start=True,
                            stop=True,
                            tile_position=(0, 0),
                        )
                        nc.tensor.matmul(
                            st_ps[:, 1, :],
                            kT[Dh:128, fc, jsl],
                            qT[Dh:128, fc, isl],
                            start=True,
                            stop=True,
                            tile_position=(64, 0),
                        )
                        e = epool.tile([128, 2, IB], BF16, tag="E", name="e")
                        nc.scalar.activation(
                            e[:], st_ps[:], mybir.ActivationFunctionType.Exp
                        )
                        for t in range(2):
                            nc.tensor.matmul(
                                ats[t][:],
                                vA[:, jc, 2 * fc + t, :],
                                e[:, t, :],
                                start=(jc == 0),
                                stop=(jc == NST - 1),
                            )
                    atcs = []
                    for t in range(2):
                        atc = spool.tile([Dh + 1, IB], F32, tag="atc", name="atc")
                        nc.vector.tensor_copy(atc[:], ats[t][:])
                        atcs.append(atc)
                    for t in range(2):
                        po = t * Dh
                        rz = spool.tile([1, IB], F32, tag="rz", name="rz")
                        nc.vector.reciprocal(rz[:], atcs[t][Dh : Dh + 1, :])
                        bc = spool.tile([Dh, IB], F32, tag="bc", name="bc")
                        nc.gpsimd.partition_broadcast(bc[:], rz[:])
                        nc.gpsimd.tensor_tensor(
                            aT[po : po + Dh, fc, isl],
                            atcs[t][0:Dh, :],
                            bc[:],
                            mybir.AluOpType.mult,
                        )

                if ib + 1 < NIB:
                    q_proj(ib + 1)
                if ib > 0:
                    out_proj(ib - 1)
            out_proj(NIB - 1)

    nc.compile()
    return nc


def kernel(key, query, value, Wq, bq, Wk, bk, Wv, bv, Wo, bo):
    global last_exec_time_ns, last_results, _built
    key = np.asarray(key, np.float32)
    query = np.asarray(query, np.float32)
    value = np.asarray(value, np.float32)
    Wq = np.asarray(Wq, np.float32)
    Wk = np.asarray(Wk, np.float32)
    Wv = np.asarray(Wv, np.float32)
    Wo = np.asarray(Wo, np.float32)
    bq = np.asarray(bq, np.float32)
    bv = np.asarray(bv, np.float32)
    bo = np.asarray(bo, np.float32)

    s = 1.0 / math.sqrt(Dh)
    xqT = [np.ascontiguousarray(query[b].T) for b in range(B)]
    xkT = [np.ascontiguousarray(key[b].T) for b in range(B)]
    xvT = [np.ascontiguousarray(value[b].T) for b in range(B)]
    WqTs = np.ascontiguousarray(Wq.T) * s
    WkT = np.ascontiguousarray(Wk.T)
    WvT = np.ascontiguousarray(Wv.T)
    WoT = np.ascontiguousarray(Wo.T)

    in_maps = []
    for c in range(NCORES):
        b, g = c // GPB, c % GPB
        fsl = slice(g * F, (g + 1) * F)
        in_maps.append(
            {
                "xq": xqT[b],
                "xk": xkT[b],
                "xv": xvT[b],
                "wqt": np.ascontiguousarray(WqTs[:, fsl]),
                "wkt": np.ascontiguousarray(WkT[:, fsl]),
                "wvt": np.ascontiguousarray(WvT[:, fsl]),
                "wot": np.ascontiguousarray(WoT[fsl, :]).astype(ml_dtypes.bfloat16),
                "bqv": np.ascontiguousarray((bq[fsl] * s).reshape(1, F)),
            }
        )

    if _built is None:
        _built = _build()
    trace = bool(int(os.environ.get("KERNEL_TRACE", "0")))
    res = run_bass_kernel_spmd(
        _built, in_maps, core_ids=list(range(NCORES)), trace=trace
    )
    last_exec_time_ns = res.exec_time_ns
    last_results = res
    parts = [r["out"] for r in res.results]

    outs = np.stack(
        [sum(parts[b * GPB + g] for g in range(GPB)) for b in range(B)]
    ).astype(np.float32)
    outs += (bv @ Wo.T + bo)[None, None, :]
    return outs


# revision 20
# speedup vs baseline: 1.5395x; 1.4444x over previous
"""Multi-head attention (B=2, S=2048, E=1024, H=16) on 8 TRN2 NeuronCores.

Sharding: batch x head-group. Core c handles batch b = c // 4 and the
4 heads (256 features) of group g = c % 4. Each core computes its
q/k/v projections (column-sharded weights), transposed-layout attention
(scores kept as [j, i] so exp(scores) feeds the P@V matmul directly as
the moving operand), and a partial output projection against its row
slice of Wo^T. Host sums the 4 partials per batch and folds in the
bv/bo biases (exact: softmax rows sum to 1, so bv contributes
bv @ Wo.T; bk is softmax-invariant and dropped; bq and the 1/sqrt(Dh)
scale are folded into Wq/bq host-side).

Matmuls run as float32r (full-rate fp32 on the PE with TF32-like
mantissa rounding); softmax runs unshifted (scores are O(5), safe in
fp32) with the row sum obtained for free by augmenting V with a ones
column. exp() runs on the scalar engine out of PSUM; normalization is a
DVE multiply against a gpsimd partition-broadcast of 1/Z.
"""

import math
import os

import ml_dtypes
import numpy as np

import concourse.bass as bass
from concourse import bacc
import concourse.mybir as mybir
import concourse.tile as tile
from concourse.bass_utils import run_bass_kernel_spmd

B, S, E, H = 2, 2048, 1024, 16
Dh = E // H  # 64
NCORES = 8
GPB = NCORES // B  # head-groups (cores) per batch
HPC = H // GPB  # heads per core
F = HPC * Dh  # 256 features per core
FC = F // 128  # 2 f-chunks
EC = E // 128  # 8 e-chunks
SB = 512  # s-block (projection/out-proj N)
NSB = S // SB
NST = S // 128  # 16 s-tiles / j-chunks
IB = 512  # attention i-block
NIB = S // IB
F32 = mybir.dt.float32
F32R = mybir.dt.float32r
BF16 = mybir.dt.bfloat16

# exec time (ns) of the last traced run; test.py reads this.
last_exec_time_ns = None
last_results = None

_built = None


def _build():
    nc = bacc.Bacc()
    xq = nc.dram_tensor("xq", [E, S], F32R, kind="ExternalInput")  # query[b].T
    xk = nc.dram_tensor("xk", [E, S], F32R, kind="ExternalInput")  # key[b].T
    xv = nc.dram_tensor("xv", [E, S], F32R, kind="ExternalInput")  # value[b].T
    wqt = nc.dram_tensor("wqt", [E, F], F32R, kind="ExternalInput")
    wkt = nc.dram_tensor("wkt", [E, F], F32R, kind="ExternalInput")
    wvt = nc.dram_tensor("wvt", [E, F], F32R, kind="ExternalInput")
    wot = nc.dram_tensor("wot", [F, E], BF16, kind="ExternalInput")
    bqv = nc.dram_tensor("bqv", [1, F], F32R, kind="ExternalInput")
    out = nc.dram_tensor("out", [S, E], F32, kind="ExternalOutput")

    with tile.TileContext(nc) as tc:
        with (
            tc.tile_pool(name="wpool", bufs=1) as wpool,
            tc.tile_pool(name="persist", bufs=1) as persist,
            tc.tile_pool(name="xpool", bufs=3) as xpool,
            tc.tile_pool(name="epool", bufs=8) as epool,
            tc.tile_pool(name="spool", bufs=4) as spool,
            tc.tile_pool(name="opool", bufs=2) as opool,
            tc.tile_pool(name="pp", bufs=2, space="PSUM") as pp,
            tc.tile_pool(name="stp", bufs=2, space="PSUM") as stp,
            tc.tile_pool(name="atp", bufs=2, space="PSUM") as atp,
        ):
            wq_sb = wpool.tile([128, EC, F], F32R, name="wq_sb")
            nc.sync.dma_start(wq_sb[:], wqt.rearrange("(ec p) f -> p ec f", p=128))
            wk_sb = wpool.tile([128, EC, F], F32R, name="wk_sb")
            nc.sync.dma_start(wk_sb[:], wkt.rearrange("(ec p) f -> p ec f", p=128))
            wv_sb = wpool.tile([128, EC, F], F32R, name="wv_sb")
            nc.sync.dma_start(wv_sb[:], wvt.rearrange("(ec p) f -> p ec f", p=128))
            wo_sb = wpool.tile([128, FC, E], BF16, name="wo_sb")
            nc.sync.dma_start(wo_sb[:], wot.rearrange("(fc p) n -> p fc n", p=128))
            bq_sb = wpool.tile([1, F], F32R, name="bq_sb")
            nc.sync.dma_start(bq_sb[:], bqv[:, :])
            ones1 = wpool.tile([1, SB], F32R, name="ones1")
            nc.vector.memset(ones1[:].bitcast(F32), 1.0)

            qT = persist.tile([128, FC, S], BF16, name="qT")
            kT = persist.tile([128, FC, S], BF16, name="kT")
            vA = persist.tile([128, NST, HPC, Dh + 1], BF16, name="vA")
            aT = persist.tile([128, FC, S], BF16, name="aT")
            nc.vector.memset(vA[:, :, :, Dh : Dh + 1].bitcast(mybir.dt.uint16), 0x3F80)

            # --- q projection helper (emitted per i-block, earliest) ---
            xq_r = xq.rearrange("(ec p) s -> p ec s", p=128)

            def q_proj(ib):
                isl = slice(ib * IB, (ib + 1) * IB)
                xt = xpool.tile([128, EC, SB], F32R, tag="xT", name="xt")
                nc.sync.dma_start(xt[:], xq_r[:, :, isl])
                for fc in range(FC):
                    ps = pp.tile([128, SB], F32, tag="ps", name="ps")
                    for ec in range(EC):
                        nc.tensor.matmul(
                            ps[:],
                            wq_sb[:, ec, fc * 128 : (fc + 1) * 128],
                            xt[:, ec, :],
                            start=(ec == 0),
                            stop=False,
                        )
                    nc.tensor.matmul(
                        ps[:],
                        bq_sb[0:1, fc * 128 : (fc + 1) * 128],
                        ones1[0:1, :],
                        start=False,
                        stop=True,
                    )
                    nc.scalar.activation(
                        qT[:, fc, isl], ps[:], mybir.ActivationFunctionType.Copy
                    )

            q_proj(0)

            # --- k projection: kT[f, s] = Wk.T slice @ x.T ---
            xk_r = xk.rearrange("(ec p) s -> p ec s", p=128)
            for sb in range(NSB):
                xt = xpool.tile([128, EC, SB], F32R, tag="xT", name="xt")
                nc.sync.dma_start(xt[:], xk_r[:, :, sb * SB : (sb + 1) * SB])
                for fc in range(FC):
                    ps = pp.tile([128, SB], F32, tag="ps", name="ps")
                    for ec in range(EC):
                        nc.tensor.matmul(
                            ps[:],
                            wk_sb[:, ec, fc * 128 : (fc + 1) * 128],
                            xt[:, ec, :],
                            start=(ec == 0),
                            stop=(ec == EC - 1),
                        )
                    nc.vector.tensor_copy(kT[:, fc, sb * SB : (sb + 1) * SB], ps[:])

            # --- v projection: v[s, f] (s on partitions), ones column at f=Dh ---
            xv_r = xv.rearrange("(ec p) s -> p ec s", p=128)
            for sb in range(NSB):
                xt = xpool.tile([128, EC, SB], F32R, tag="xT", name="xt")
                nc.sync.dma_start(xt[:], xv_r[:, :, sb * SB : (sb + 1) * SB])
                for st in range(SB // 128):
                    jt = sb * (SB // 128) + st
                    psv = pp.tile([128, F], F32, tag="ps", name="psv")
                    for ec in range(EC):
                        nc.tensor.matmul(
                            psv[:],
                            xt[:, ec, st * 128 : (st + 1) * 128],
                            wv_sb[:, ec, :],
                            start=(ec == 0),
                            stop=(ec == EC - 1),
                        )
                    nc.vector.tensor_copy(
                        vA[:, jt, :, 0:Dh],
                        psv[:].rearrange("p (h d) -> p h d", h=HPC),
                    )

            # --- per i-block: attention (head-pairs packed into PE row
            #     groups); q-proj of the next block leads each segment and
            #     out-proj trails by one block to keep the PE fed while the
            #     DVE/gpsimd normalize chain drains ---

            def out_proj(ib):
                for st in range(ib * (IB // 128), (ib + 1) * (IB // 128)):
                    ob = opool.tile([128, E], F32, tag="ob", name="ob")
                    for n2 in range(E // SB):
                        pso = pp.tile([128, SB], F32, tag="ps", name="pso")
                        for fc in range(FC):
                            nc.tensor.matmul(
                                pso[:],
                                aT[:, fc, st * 128 : (st + 1) * 128],
                                wo_sb[:, fc, n2 * SB : (n2 + 1) * SB],
                                start=(fc == 0),
                                stop=(fc == FC - 1),
                            )
                        nc.vector.tensor_copy(ob[:, n2 * SB : (n2 + 1) * SB], pso[:])
                    nc.sync.dma_start(out[st * 128 : (st + 1) * 128, :], ob[:])

            for ib in range(NIB):
                isl = slice(ib * IB, (ib + 1) * IB)
                for fc in range(FC):
                    ats = [
                        atp.tile([Dh + 1, IB], F32, tag="at", name=f"at{t}")
                        for t in range(2)
                    ]
                    for jc in range(NST):
                        jsl = slice(jc * 128, (jc + 1) * 128)
                        st_ps = stp.tile([128, 2, IB], F32, tag="st", name="st")
                        nc.tensor.matmul(
                            st_ps[:, 0, :],
                            kT[0:Dh, fc, jsl],
                            qT[0:Dh, fc, isl],
                            start=True,
                            stop=True,
                            tile_position=(0, 0),
                        )
                        nc.tensor.matmul(
                            st_ps[:, 1, :],
                            kT[Dh:128, fc, jsl],
                            qT[Dh:128, fc, isl],
                            start=True,
                            stop=True,
                            tile_position=(64, 0),
                        )
                        e = epool.tile([128, 2, IB], BF16, tag="E", name="e")
                        nc.scalar.activation(
                            e[:], st_ps[:], mybir.ActivationFunctionType.Exp
                        )
                        for t in range(2):
                            nc.tensor.matmul(
                                ats[t][:],
                                vA[:, jc, 2 * fc + t, :],
                                e[:, t, :],
                                start=(jc == 0),
                                stop=(jc == NST - 1),
                            )
                    atcs = []
                    for t in range(2):
                        atc = spool.tile([Dh + 1, IB], F32, tag="atc", name="atc")
                        nc.vector.tensor_copy(atc[:], ats[t][:])
                        atcs.append(atc)
                    for t in range(2):
                        po = t * Dh
                        rz = spool.tile([1, IB], F32, tag="rz", name="rz")
                        nc.vector.reciprocal(rz[:], atcs[t][Dh : Dh + 1, :])
                        bc = spool.tile([Dh, IB], F32, tag="bc", name="bc")
                        nc.gpsimd.partition_broadcast(bc[:], rz[:])
                        nc.vector.tensor_tensor(
                            aT[po : po + Dh, fc, isl],
                            atcs[t][0:Dh, :],
                            bc[:],
                            mybir.AluOpType.mult,
                        )

                if ib + 1 < NIB:
                    q_proj(ib + 1)
                if ib > 0:
                    out_proj(ib - 1)
            out_proj(NIB - 1)

    nc.compile()
    return nc


def kernel(key, query, value, Wq, bq, Wk, bk, Wv, bv, Wo, bo):
    global last_exec_time_ns, last_results, _built
    key = np.asarray(key, np.float32)
    query = np.asarray(query, np.float32)
    value = np.asarray(value, np.float32)
    Wq = np.asarray(Wq, np.float32)
    Wk = np.asarray(Wk, np.float32)
    Wv = np.asarray(Wv, np.float32)
    Wo = np.asarray(Wo, np.float32)
    bq = np.asarray(bq, np.float32)
    bv = np.asarray(bv, np.float32)
    bo = np.asarray(bo, np.float32)

    s = 1.0 / math.sqrt(Dh)
    xqT = [np.ascontiguousarray(query[b].T) for b in range(B)]
    xkT = [np.ascontiguousarray(key[b].T) for b in range(B)]
    xvT = [np.ascontiguousarray(value[b].T) for b in range(B)]
    WqTs = np.ascontiguousarray(Wq.T) * s
    WkT = np.ascontiguousarray(Wk.T)
    WvT = np.ascontiguousarray(Wv.T)
    WoT = np.ascontiguousarray(Wo.T)

    in_maps = []
    for c in range(NCORES):
        b, g = c // GPB, c % GPB
        fsl = slice(g * F, (g + 1) * F)
        in_maps.append(
            {
                "xq": xqT[b],
                "xk": xkT[b],
                "xv": xvT[b],
                "wqt": np.ascontiguousarray(WqTs[:, fsl]),
                "wkt": np.ascontiguousarray(WkT[:, fsl]),
                "wvt": np.ascontiguousarray(WvT[:, fsl]),
                "wot": np.ascontiguousarray(WoT[fsl, :]).astype(ml_dtypes.bfloat16),
                "bqv": np.ascontiguousarray((bq[fsl] * s).reshape(1, F)),
            }
        )

    if _built is None:
        _built = _build()
    trace = bool(int(os.environ.get("KERNEL_TRACE", "0")))
    res = run_bass_kernel_spmd(
        _built, in_maps, core_ids=list(range(NCORES)), trace=trace
    )
    last_exec_time_ns = res.exec_time_ns
    last_results = res
    parts = [r["out"] for r in res.results]

    outs = np.stack(
        [sum(parts[b * GPB + g] for g in range(GPB)) for b in range(B)]
    ).astype(np.float32)
    outs += (bv @ Wo.T + bo)[None, None, :]
    return outs
